# revision 127
# baseline (speedup 1.0000x reference)
# kernel.py — Trainium2 local-attention encoder layer (fp8 DoubleRow)
#
# Sharding: batch(4) x seq-half(2) over 8 cores; each core handles 1024
# query tokens with a 1280-slot kv window (128-token halo each side).
#
# Per-core structure: QKV/out-proj/FFN/FFN2 run as fp8e4m3 DoubleRow
# matmuls; scores/AV stay bf16 for accuracy. Residual adds are folded into
# PSUM via identity-matmuls. Key scheduling points (engine queues are
# in-order, so emission order is the schedule):
#  - QKV evictions merged into [128,1024] PSUM tiles (halves per-op cost);
#    Q is projected packed (plain wq) and expanded to the per-head
#    zero-padded layout by Pool copies/memsets, halving ACT/DVE traffic.
#  - xf8 arrives in two DMA chunks so V/K/Q matmuls start early.
#  - out-proj(0) is hoisted before the last attnT flush to fill the PE
#    drain of the attention tail; out-proj(1) is split by token halves so
#    half 0 (flushes 4,5) also runs pre-flush(7); late flush-muls ride the
#    idle ACT engine. LN stats use fp8+DoubleRow for the mean-of-squares;
#    LN chains subtract the mean early so only the rstd multiply trails
#    the reciprocal.
#  - FFN2 ib1 runs token-half-major so the final LN2 tail is one half's
#    chain; its y ops are two wide strided DVE ops in 2x bf16 mode.
#  - QKV PSUM ring is 4 deep (warmup tile shares the tag) so the in-order
#    PE queue drains QKV sooner; out-proj zb evictions split ACT/DVE.
# Band mask multiplies only the two triangular key tiles (jt 0/2) on DVE.
# Output is bf16, cast to fp32 on host.
import functools
import os
import sys

import numpy as np

sys.path.insert(0, "/opt/trn_rl_repo")

import ml_dtypes  # noqa: E402

D = 512        # d_model
H = 8          # heads
DH = 64        # head dim
WIN = 128      # attention window
F = 2048       # ff dim
B = 4
S = 2048
EPS = 1e-5
NCORES = 8
NQ = 1024      # query tokens per core
KV = 1280      # kv slots per core (incl halo)
NKT = KV // 128
NQT = NQ // 128
ET = D // 128
FT = F // 128

BF16 = ml_dtypes.bfloat16
F8 = ml_dtypes.float8_e4m3

_last_results = None  # stash for test.py


def _build_program():
    import concourse.bass as bass
    import concourse.tile as tile
    from concourse import bacc, mybir

    dt = mybir.dt
    f32, bf16, f8 = dt.float32, dt.bfloat16, dt.float8e4
    AF = mybir.ActivationFunctionType
    OP = mybir.AluOpType
    DR = mybir.MatmulPerfMode.DoubleRow
    PSUM = bass.MemorySpace.PSUM

    nc = bacc.Bacc("TRN2", target_bir_lowering=False, debug=False,
                   num_devices=NCORES)

    # ---- DRAM I/O ----
    xf8_d = nc.dram_tensor("xf8", [D, KV], f8, kind="ExternalInput")
    xtb_d = nc.dram_tensor("xtb", [D, KV], bf16, kind="ExternalInput")
    wqk_d = nc.dram_tensor("wqkT", [D, 2 * D], f8, kind="ExternalInput")
    wv_d = nc.dram_tensor("wvT", [D, D], f8, kind="ExternalInput")
    wo_d = nc.dram_tensor("woT", [128, ET * D], f8, kind="ExternalInput")
    w1_d = nc.dram_tensor("w1T", [128, ET * F], f8, kind="ExternalInput")
    w2_d = nc.dram_tensor("w2T", [128, FT * D], f8, kind="ExternalInput")
    b1_d = nc.dram_tensor("b1t", [128, FT], f32, kind="ExternalInput")
    b2_d = nc.dram_tensor("b2t", [128, ET], f32, kind="ExternalInput")
    msk_d = nc.dram_tensor("masks", [128, NQT * 256], bf16, kind="ExternalInput")
    idbo_d = nc.dram_tensor("idbo", [128, 128], bf16, kind="ExternalInput")
    idt_d = nc.dram_tensor("idt", [128, 128], bf16, kind="ExternalInput")
    idb2_d = nc.dram_tensor("idb2", [128, 128], bf16, kind="ExternalInput")
    outT_d = nc.dram_tensor("outT", [D, NQ], bf16, kind="ExternalOutput")
    DBG = bool(int(os.environ.get("TRN_DEBUG_DUMP", "0")))
    if DBG:
        dbg_zb = nc.dram_tensor("dbg_zb", [128, ET * 512], bf16,
                                kind="ExternalOutput")
        dbg_y1 = nc.dram_tensor("dbg_y1", [128, ET * 512], bf16,
                                kind="ExternalOutput")
        dbg_hs = nc.dram_tensor("dbg_hs", [128, FT * 512], bf16,
                                kind="ExternalOutput")
        dbg_at = nc.dram_tensor("dbg_at", [128, ET * 512], f8,
                                kind="ExternalOutput")
        dbg_q = nc.dram_tensor("dbg_q", [128, H * 512], bf16,
                               kind="ExternalOutput")
        dbg_k = nc.dram_tensor("dbg_k", [128, ET * 512], bf16,
                               kind="ExternalOutput")

    def sub_ap(t, extra_off, dims):
        return bass.AP(tensor=t.tensor, offset=t.offset + extra_off,
                       ap=[t.ap[0]] + dims)

    from contextlib import ExitStack
    with tile.TileContext(nc) as tc, ExitStack() as _es:
        _es.enter_context(nc.allow_low_precision(
            reason="bf16/fp8 residual stream is within the 2e-2 tolerance"))
        persist_cm = tc.tile_pool(name="persist", bufs=1)
        persist = persist_cm.__enter__()

        xf8s = persist.tile([128, ET * KV], f8, tag="xf8s")
        xtbs = persist.tile([128, ET * KV], bf16, tag="xtbs")
        wo = persist.tile([128, ET * D], f8, tag="wo")
        w1 = persist.tile([128, ET * F], f8, tag="w1")
        w2 = persist.tile([128, FT * D], f8, tag="w2")
        b1s = persist.tile([128, FT], f32, tag="b1s")
        b2s = persist.tile([128, ET], f32, tag="b2s")
        idbo = persist.tile([128, 128], bf16, tag="idbo")
        idb2 = persist.tile([128, 128], bf16, tag="idb2")
        onesb = persist.tile([128, 128], bf16, tag="onesb")
        onesf8 = persist.tile([128, 128], f8, tag="onesf8")
        epsb = persist.tile([128, 1], f32, tag="epsb")
        zcol = persist.tile([128, 1], f32, tag="zcol")
        attnT = [persist.tile([128, ET * 512], f8, tag=f"attnT{i}",
                              name=f"attnT{i}") for i in range(2)]
        zbt = [persist.tile([128, ET * 512], bf16, tag=f"zbt{i}",
                            name=f"zbt{i}") for i in range(2)]
        # squares in fp8: only feed the mean-of-squares reduction, which the
        # DoubleRow stats matmul then does in half the PE cycles
        zsqt = [persist.tile([128, ET * 512], f8, tag=f"zsqt{i}",
                             name=f"zsqt{i}") for i in range(2)]
        y1b = [persist.tile([128, ET * 512], bf16, tag=f"y1b{i}",
                            name=f"y1b{i}") for i in range(2)]
        y1f8 = [persist.tile([128, ET * 512], f8, tag=f"y1f8_{i}",
                             name=f"y1f8_{i}") for i in range(2)]
        y2t = [persist.tile([128, ET * 512], bf16, tag=f"y2t{i}",
                            name=f"y2t{i}") for i in range(2)]
        hs = [persist.tile([128, FT * 512], f8, tag=f"hs{i}",
                           name=f"hs{i}") for i in range(2)]
        musq = [persist.tile([128, 512], f32, tag=f"musq{i}",
                             name=f"musq{i}") for i in range(2)]
        vart = [persist.tile([128, 512], f32, tag=f"vart{i}",
                             name=f"vart{i}") for i in range(2)]
        stdt = [persist.tile([128, 512], f32, tag=f"stdt{i}",
                             name=f"stdt{i}") for i in range(2)]
        rstdb = [persist.tile([128, 512], bf16, tag=f"rstdb{i}",
                              name=f"rstdb{i}") for i in range(2)]
        cmub = [persist.tile([128, 512], bf16, tag=f"cmub{i}",
                             name=f"cmub{i}") for i in range(2)]

        nc.vector.memset(onesb[:], 1.0 / D)
        nc.gpsimd.memset(onesf8[:], 1.0 / D)
        nc.vector.memset(epsb[:], EPS)
        nc.vector.memset(zcol[:], 0.0)
        tblx = persist.tile([128, 1], f32, tag="tblx")
        # first ACT op: pulls in the exp_and_friends table during the DMA
        # head so neither p1's copies nor exp(qt0) pay the 1.3us load
        nc.scalar.activation(tblx[:], epsb[:], AF.Exp)

        p1_cm = tc.tile_pool(name="p1sb", bufs=1)
        p1sb = p1_cm.__enter__()
        wqk = p1sb.tile([128, ET * 2 * D], f8, tag="wqk")
        wv = p1sb.tile([128, ET * D], f8, tag="wv")
        masks = p1sb.tile([128, NQT * 256], bf16, tag="masks")
        idt = p1sb.tile([128, 128], bf16, tag="idt")
        qs = [p1sb.tile([128, H * 512], bf16, tag=f"qs{c}", name=f"qs{c}")
              for c in range(2)]
        ks = [p1sb.tile([128, ET * csz], bf16, tag=f"ks{c}", name=f"ks{c}")
              for c, csz in ((0, 512), (1, 512), (2, 256))]
        vs = [p1sb.tile([128, n * 520], bf16, tag=f"vs{c}", name=f"vs{c}")
              for c, n in ((0, 4), (1, 4), (2, 2))]
        probs_cm = tc.tile_pool(name="probs_pool", bufs=5)
        probs_pool = probs_cm.__enter__()
        asm_cm = tc.tile_pool(name="attn_sm", bufs=3)
        attn_sm = asm_cm.__enter__()

        # ---- batched DMAs, in consumption order ----
        _xo = xf8s[:].rearrange("p (et kv) -> p et kv", et=ET)
        _xi = xf8_d.rearrange("(et p) kv -> p et kv", p=128)
        nc.sync.dma_start(
            out=bass.AP(tensor=_xo.tensor, offset=_xo.offset,
                        ap=[_xo.ap[0], _xo.ap[1], [1, 640]]),
            in_=bass.AP(tensor=_xi.tensor, offset=_xi.offset,
                        ap=[_xi.ap[0], _xi.ap[1], [1, 640]]))
        nc.sync.dma_start(
            out=wv[:].rearrange("p (et d) -> p et d", et=ET),
            in_=wv_d.rearrange("(et p) d -> p et d", p=128))
        nc.sync.dma_start(
            out=bass.AP(tensor=_xo.tensor, offset=_xo.offset + 640,
                        ap=[_xo.ap[0], _xo.ap[1], [1, KV - 640]]),
            in_=bass.AP(tensor=_xi.tensor, offset=_xi.offset + 640,
                        ap=[_xi.ap[0], _xi.ap[1], [1, KV - 640]]))
        nc.sync.dma_start(
            out=wqk[:].rearrange("p (et d) -> p et d", et=ET),
            in_=wqk_d.rearrange("(et p) d -> p et d", p=128))
        nc.sync.dma_start(out=masks[:], in_=msk_d[:])
        nc.sync.dma_start(out=idbo[:], in_=idbo_d[:])
        nc.sync.dma_start(out=idt[:], in_=idt_d[:])
        nc.sync.dma_start(out=idb2[:], in_=idb2_d[:])
        nc.sync.dma_start(
            out=xtbs[:].rearrange("p (et kv) -> p et kv", et=ET),
            in_=xtb_d.rearrange("(et p) kv -> p et kv", p=128))
        nc.sync.dma_start(out=wo[:], in_=wo_d[:])
        nc.sync.dma_start(out=w1[:], in_=w1_d[:])
        nc.sync.dma_start(out=b1s[:], in_=b1_d[:])
        nc.sync.dma_start(out=w2[:], in_=w2_d[:])
        nc.sync.dma_start(out=b2s[:], in_=b2_d[:])

        # greedy cost-balanced eviction engine choice
        _eng_load = {"dve": 0.0, "act": 0.0, "pool": 0.0}

        def _evict_cost(eng, width):
            if eng == "dve":
                return width * 1.0417 + 185
            if eng == "act":
                return width * 0.8333 + 185
            return width * 1.39 + 95

        def evict(out_ap, in_ap, width=512, exclude=()):
            # gpsimd cannot read PSUM on hw; PSUM evictions go DVE/ACT only
            cands = [e for e in ("dve", "act") if e not in exclude]
            e = min(cands, key=lambda g: _eng_load[g] + _evict_cost(g, width))
            _eng_load[e] += _evict_cost(e, width)
            if e == "dve":
                nc.vector.tensor_copy(out_ap, in_ap)
            elif e == "act":
                nc.scalar.activation(out_ap, in_ap, AF.Copy)
            else:
                nc.gpsimd.tensor_copy(out_ap, in_ap)

        # ================= Phase 1: QKV (fp8 DoubleRow) =================
        psA_cm = tc.tile_pool(name="psA", bufs=6, space=PSUM)
        psA = psA_cm.__enter__()

        # warm-up: keep PE busy through the HAM/p-state ramp while the
        # first DMAs land (cost model: 3us of continuous work -> full clock)
        pwu = psA.tile([128, 128], f32, tag="pwu", name="pwu", bufs=1)
        for _ in range(28):
            nc.tensor.matmul(pwu[:, :128], onesb[:], onesb[:],
                             start=True, stop=True)

        def dr_pair(t, base_off, pair_stride, n):
            return sub_ap(t, base_off, [[pair_stride, 2], [1, n]])

        # QKV evictions are merged in [128,1024] PSUM tiles (2 banks) to
        # halve the per-op fixed cost on ACT/DVE
        def emit_v2(t0, exclude=()):
            c, ti = (0, t0) if t0 < 4 else (1, t0 - 4) if t0 < 8 else (2, t0 - 8)
            pv = psA.tile([128, 1024], f32, tag="pq2", name=f"pv{t0}", bufs=3)
            for i in range(2):
                for p in range(2):
                    nc.tensor.matmul(
                        pv[:, i * 512:(i + 1) * 512],
                        dr_pair(xf8s, (2 * p) * KV + (t0 + i) * 128, KV, 128),
                        dr_pair(wv, (2 * p) * D, D, D),
                        start=(p == 0), stop=(p == 1), perf_mode=DR)
            vt = vs[c][:, ti * 520:(ti + 2) * 520]
            evict(sub_ap(vt, 0, [[520, 2], [65, 8], [1, 64]]),
                  pv[:].rearrange("p (t h d) -> p t h d", t=2, h=8),
                  width=1024, exclude=exclude)
            nc.gpsimd.memset(sub_ap(vt, 64, [[520, 2], [65, 8]]), 256.0)

        def emit_k(c, lo, hi, exclude=()):
            w = hi - lo
            g = 1024 // w
            for f0 in range(0, ET, g):
                pk = psA.tile([128, 1024], f32, tag="pq2",
                              name=f"pk{c}_{f0}", bufs=4)
                for i in range(g):
                    for p in range(2):
                        nc.tensor.matmul(
                            pk[:, i * w:(i + 1) * w],
                            dr_pair(wqk, (2 * p) * 2 * D + D
                                    + (f0 + i) * 128, 2 * D, 128),
                            dr_pair(xf8s, (2 * p) * KV + lo, KV, w),
                            start=(p == 0), stop=(p == 1), perf_mode=DR)
                evict(ks[c][:, f0 * w:(f0 + g) * w], pk[:],
                      width=1024, exclude=exclude)

        def emit_q(c, lo, hi):
            # packed projection: plain wq, head pair (2ft, 2ft+1) stacked in
            # the 128 partition rows — half the matmuls and evict volume of
            # the zero-padded layout. Pool then splits each pair into the
            # per-head zero-padded qs blocks (copy + 2 memsets per pair).
            for f0 in range(0, ET, 2):
                pq = psA.tile([128, 1024], f32, tag="pq2",
                              name=f"pq{c}_{f0}", bufs=4)
                for i in range(2):
                    for p in range(2):
                        nc.tensor.matmul(
                            pq[:, i * 512:(i + 1) * 512],
                            dr_pair(wqk, (2 * p) * 2 * D + (f0 + i) * 128,
                                    2 * D, 128),
                            dr_pair(xf8s, (2 * p) * KV + lo, KV, hi - lo),
                            start=(p == 0), stop=(p == 1), perf_mode=DR)
                evict(sub_ap(qs[c], f0 * 1024, [[1024, 2], [1, 512]]),
                      pq[:].rearrange("p (t n) -> p t n", t=2), width=1024)
                for ft in (f0, f0 + 1):
                    base = ft * 1024
                    nc.gpsimd.memset(qs[c][0:64, base + 512:base + 1024], 0.0)
                    nc.gpsimd.tensor_copy(
                        qs[c][64:128, base + 512:base + 1024],
                        qs[c][64:128, base:base + 512])
                    nc.gpsimd.memset(qs[c][64:128, base:base + 512], 0.0)

        for t0 in range(0, 6, 2):
            emit_v2(t0)
        emit_k(0, 0, 512)
        emit_q(0, 128, 640)
        emit_k(1, 512, 1024)
        emit_q(1, 640, 1152)
        emit_k(2, 1024, 1280)
        emit_v2(6)

        # ============ Phase 2: attention (+ overlapped out-proj ib0) ========
        psA_cm.__exit__(None, None, None)
        psB_cm = tc.tile_pool(name="psB", bufs=1, space=PSUM)
        psB = psB_cm.__enter__()

        def outproj_evict(ib, zb_eng, zsq_eng):
            # et2 pairs share one [128,1024] PSUM region (first 1024 cols of
            # a sblk-ring tile) so zb/zsq evictions run as single wide ops
            for e0 in range(0, ET, 2):
                po = psB.tile([128, 1536], f32, tag="sblk",
                              name=f"po{ib}_{e0}", bufs=2)
                for i in range(2):
                    et2 = e0 + i
                    for p in range(2):
                        nc.tensor.matmul(
                            po[:, i * 512:(i + 1) * 512],
                            dr_pair(wo, (2 * p) * D + et2 * 128, D, 128),
                            dr_pair(attnT[ib], (2 * p) * 512, 512, 512),
                            start=(p == 0), stop=False, perf_mode=DR)
                    # residual: + x (bf16 identity matmul)
                    nc.tensor.matmul(
                        po[:, i * 512:(i + 1) * 512], idbo[:],
                        xtbs[:, et2 * KV + 128 + ib * 512:
                             et2 * KV + 128 + ib * 512 + 512],
                        start=False, stop=True)
                zsl = zbt[ib][:, e0 * 512:(e0 + 2) * 512]
                qsl = zsqt[ib][:, e0 * 512:(e0 + 2) * 512]
                eng = zb_eng if zb_eng != "mix" else \
                    ("act" if e0 == 0 else "dve")
                if eng == "dve":
                    nc.vector.tensor_scalar_mul(zsl, po[:, :1024],
                                                1.0 / 16384.0)
                else:
                    nc.scalar.activation(zsl, po[:, :1024], AF.Copy,
                                         scale=1.0 / 16384.0)
                if zsq_eng == "act":
                    nc.scalar.activation(qsl, po[:, :1024], AF.Square,
                                         scale=1.0 / 16384.0)
                elif zsq_eng == "dve":
                    nc.vector.tensor_tensor(qsl, zsl, zsl, OP.mult)
                elif zsq_eng == "pool":
                    nc.gpsimd.tensor_tensor(qsl, zsl, zsl, OP.mult)

        def flush_transpose(fq, attn_i):
            fib, fibo = fq // 4, (fq % 4) * 128
            pt = psB.tile([128, 512], bf16, tag="pt", bufs=1)
            for et in range(ET):
                nc.tensor.transpose(pt[:, et * 128:(et + 1) * 128],
                                    attn_i[:, et * 128:(et + 1) * 128],
                                    idt[:])
            out = sub_ap(attnT[fib][:], fibo, [[512, ET], [1, 128]])
            in_ = pt[:].rearrange("p (e i) -> p e i", e=ET)
            if fq >= 5:
                # late flushes ride the idle ACT engine so attnT[1] lands
                # sooner than the congested DVE queue would deliver it
                nc.scalar.activation(out, in_, AF.Copy, scale=64.0)
            else:
                nc.vector.tensor_scalar_mul(out, in_, 64.0)

        def emit_scores_exp(qt):
            out = []
            for hg in range(2):
                sblk = psB.tile([128, 1536], f32, tag="sblk", bufs=2)
                for jt in range(3):
                    kt = qt + jt
                    kc, ko = (0, kt) if kt < 4 else \
                        (1, kt - 4) if kt < 8 else (2, kt - 8)
                    csz = 256 if kc == 2 else 512
                    for hh in range(4):
                        h = hg * 4 + hh
                        fo = h // 2
                        nc.tensor.matmul(
                            sblk[:, jt * 512 + hh * 128:
                                 jt * 512 + hh * 128 + 128],
                            ks[kc][:, fo * csz + ko * 128:
                                   fo * csz + ko * 128 + 128],
                            qs[qt // 4][:, h * 512 + (qt % 4) * 128:
                                        h * 512 + (qt % 4) * 128 + 128],
                            start=True, stop=True)
                probs = probs_pool.tile([128, 1536], bf16, tag="probs")
                nc.scalar.activation(probs[:], sblk[:], AF.Exp,
                                     scale=1.0 / (2048.0 * 256.0))
                out.append(probs)
            return out

        def emit_softmax_av(qt, probses):
            attn_i = attn_sm.tile([128, 512], bf16, tag="attn_i")
            recip = attn_sm.tile([128, 8], f32, tag="recip")
            for hg in range(2):
                probs = probses[hg]
                msl = masks[:, qt * 256:(qt + 1) * 256]
                meng = nc.vector
                meng.tensor_tensor(
                    sub_ap(probs, 0, [[1024, 2], [128, 4], [1, 128]]),
                    sub_ap(probs, 0, [[1024, 2], [128, 4], [1, 128]]),
                    sub_ap(msl, 0, [[128, 2], [0, 4], [1, 128]]),
                    OP.mult)
                pav = psB.tile([128, 512], f32, tag="pav", bufs=1)
                for hh in range(4):
                    for jt in range(3):
                        h = hg * 4 + hh
                        kt = qt + jt
                        vc, vo = (0, kt) if kt < 4 else \
                            (1, kt - 4) if kt < 8 else (2, kt - 8)
                        nc.tensor.matmul(
                            pav[:, hh * 65:hh * 65 + 65],
                            probs[:, jt * 512 + hh * 128:
                                  jt * 512 + hh * 128 + 128],
                            vs[vc][:, vo * 520 + h * 65:vo * 520 + h * 65 + 65],
                            start=(jt == 0), stop=(jt == 2))
                nc.vector.reciprocal(recip[:, hg * 4:hg * 4 + 4],
                                     sub_ap(pav[:], 64, [[65, 4]]))
                nc.vector.tensor_tensor(
                    attn_i[:, hg * 256:(hg + 1) * 256].rearrange(
                        "p (h d) -> p h d", h=4),
                    sub_ap(pav[:], 0, [[65, 4], [1, 64]]),
                    sub_ap(recip[:], hg * 4, [[1, 4], [0, 64]]),
                    OP.mult)
            return attn_i

        # V kv-tiles 8,9 are only read from softmax step 7 on; doing them
        # here (pav/pt slots, free until the loop's first use) shortens the
        # QKV eviction queue that gates the first scores/exp
        for vi, tt in enumerate((8, 9)):
            pvx = psB.tile([128, 512], f32,
                           tag=("pav" if vi == 0 else "pt"), bufs=1,
                           name=f"pvx{tt}")
            for p in range(2):
                nc.tensor.matmul(
                    pvx[:],
                    dr_pair(xf8s, (2 * p) * KV + tt * 128, KV, 128),
                    dr_pair(wv, (2 * p) * D, D, D),
                    start=(p == 0), stop=(p == 1), perf_mode=DR)
            vt = vs[2][:, (tt - 8) * 520:(tt - 7) * 520]
            evict(sub_ap(vt, 0, [[65, 8], [1, 64]]),
                  pvx[:].rearrange("p (h d) -> p h d", h=8), width=512)
            nc.gpsimd.memset(sub_ap(vt, 64, [[65, 8]]), 256.0)

        probs_q = {}
        atti_q = {}
        for step in range(NQT + 1):
            if step < NQT:
                probs_q[step] = emit_scores_exp(step)
            if 1 <= step < NQT + 1:
                atti_q[step - 1] = emit_softmax_av(step - 1,
                                                   probs_q.pop(step - 1))
            if 2 <= step:
                flush_transpose(step - 2, atti_q.pop(step - 2))

        # out-proj(0) slots into the PE drain of the attention tail (it
        # only needs attnT[0]). out-proj(1) is split by token halves: half 0
        # only needs flushes 4,5 so it also runs before the final flush;
        # only token-half 1 trails flush(7).
        outproj_evict(0, "act", "pool")
        po1 = [psB.tile([128, 1536], f32, tag="sblk", name=f"po1_{e0}",
                        bufs=2) for e0 in (0, 2)]

        def op1_mm(th):
            # th0 covers tokens 0-384 (flushes 4-6, all done at loop end);
            # only the last 128-token sliver trails flush(7)
            lo, w = (0, 384) if th == 0 else (384, 128)
            for j, e0 in enumerate((0, 2)):
                po = po1[j]
                for i in range(2):
                    et2 = e0 + i
                    for p in range(2):
                        nc.tensor.matmul(
                            po[:, i * 512 + lo:i * 512 + lo + w],
                            dr_pair(wo, (2 * p) * D + et2 * 128, D, 128),
                            dr_pair(attnT[1], (2 * p) * 512 + lo, 512, w),
                            start=(p == 0), stop=False, perf_mode=DR)
                    nc.tensor.matmul(
                        po[:, i * 512 + lo:i * 512 + lo + w], idbo[:],
                        xtbs[:, et2 * KV + 128 + 512 + lo:
                             et2 * KV + 128 + 512 + lo + w],
                        start=False, stop=True)

        op1_mm(0)
        flush_transpose(NQT - 1, atti_q.pop(NQT - 1))
        op1_mm(1)
        for j, e0 in enumerate((0, 2)):
            zsl = zbt[1][:, e0 * 512:(e0 + 2) * 512]
            qsl = zsqt[1][:, e0 * 512:(e0 + 2) * 512]
            # these gate the psB close: one per engine, in parallel
            if j == 0:
                nc.scalar.activation(zsl, po1[j][:, :1024], AF.Copy,
                                     scale=1.0 / 16384.0)
            else:
                nc.vector.tensor_scalar_mul(zsl, po1[j][:, :1024],
                                            1.0 / 16384.0)
            nc.gpsimd.tensor_tensor(qsl, zsl, zsl, OP.mult)
        # switch the ACT table to the sqrt set last in the transition queue
        # (reading zsqt[1] — non-negative squares — pins it after the zsq
        # evicts, off the exp stream)
        nc.scalar.activation(tblx[:], zsqt[1][:, :1], AF.Sqrt)

        stats_ps = {}

        def stats_mm(key, zt, qt_, ib, lo=0, hi=512, pool=None,
                     tags=("pmu", "psq"), tag_bufs=None):
            pmu = pool.tile([128, 512], f32, tag=tags[0],
                            name=f"pmu_{key}_{ib}", bufs=tag_bufs)
            psq = pool.tile([128, 512], f32, tag=tags[1],
                            name=f"psq_{key}_{ib}", bufs=tag_bufs)
            stats_ps[(key, ib)] = (pmu, psq)
            for et in range(ET):
                nc.tensor.matmul(pmu[:, lo:hi], onesb[:],
                                 zt[ib][:, et * 512 + lo:et * 512 + hi],
                                 start=(et == 0), stop=(et == ET - 1))
            for e2 in range(0, ET, 2):
                nc.tensor.matmul(psq[:, lo:hi],
                                 dr_pair(onesf8, 0, 0, 128),
                                 dr_pair(qt_[ib], e2 * 512 + lo, 512, hi - lo),
                                 start=(e2 == 0), stop=(e2 == ET - 2),
                                 perf_mode=DR)

        def massage(key, ib, lo=0, hi=512, sc=0):
            pmu, psq = stats_ps[(key, ib)]
            mq, vt, st = musq[sc][:, lo:hi], vart[sc][:, lo:hi], \
                stdt[sc][:, lo:hi]
            nc.scalar.activation(mq, pmu[:, lo:hi], AF.Square)
            nc.vector.scalar_tensor_tensor(vt, psq[:, lo:hi], 1.0, mq,
                                           OP.mult, OP.subtract)
            nc.scalar.activation(st, vt, AF.Sqrt, bias=epsb[:])
            nc.vector.reciprocal(rstdb[ib][:, lo:hi], st)
            nc.vector.scalar_tensor_tensor(cmub[ib][:, lo:hi], pmu[:, lo:hi],
                                           1.0, rstdb[ib][:, lo:hi],
                                           OP.mult, OP.mult)

        def ln2_chain(key, ib, lo=0, hi=512, sc=0, split_out=False,
                      ymult_eng="dve"):
            # LN with the mean-subtract pulled off the critical path:
            # y = (z - mu) runs during the variance chain, and only the
            # final rstd multiply trails the reciprocal.
            pmu, psq = stats_ps[(key, ib)]
            w = hi - lo
            mub = cmub[ib]
            nc.vector.tensor_copy(mub[:, lo:hi], pmu[:, lo:hi])
            ya = sub_ap(y2t[ib], lo, [[512, ET], [1, w]])
            za = sub_ap(zbt[ib], lo, [[512, ET], [1, w]])
            ma = sub_ap(mub, lo, [[0, ET], [1, w]])
            nc.vector.tensor_tensor(ya, za, ma, OP.subtract)
            mq, vt, st = musq[sc][:, lo:hi], vart[sc][:, lo:hi], \
                stdt[sc][:, lo:hi]
            nc.scalar.activation(mq, pmu[:, lo:hi], AF.Square)
            nc.vector.scalar_tensor_tensor(vt, psq[:, lo:hi], 1.0, mq,
                                           OP.mult, OP.subtract)
            nc.scalar.activation(st, vt, AF.Sqrt, bias=epsb[:])
            nc.vector.reciprocal(rstdb[ib][:, lo:hi], st)
            _r = outT_d.rearrange("(et p) n -> p et n", p=128)
            # split_out: two half-width mult+DMA chunks so the first DMA's
            # init overlaps the second multiply (shaves the program tail)
            chunks = ((lo, w // 2), (lo + w // 2, w // 2)) if split_out \
                else ((lo, w),)
            yeng = nc.vector if ymult_eng == "dve" else nc.gpsimd
            for clo, cw in chunks:
                ya_c = sub_ap(y2t[ib], clo, [[512, ET], [1, cw]])
                ra_c = sub_ap(rstdb[ib], clo, [[0, ET], [1, cw]])
                yeng.tensor_tensor(ya_c, ya_c, ra_c, OP.mult)
                nc.sync.dma_start(
                    out=bass.AP(tensor=_r.tensor,
                                offset=_r.offset + ib * 512 + clo,
                                ap=[_r.ap[0], _r.ap[1], [1, cw]]),
                    in_=sub_ap(y2t[ib], clo, [[512, ET], [1, cw]]))

        def ln1_chain(ib, cast_eng="act"):
            pmu, psq = stats_ps[("ln1", ib)]
            mub = cmub[ib]
            nc.vector.tensor_copy(mub[:], pmu[:])
            ya = sub_ap(y1b[ib], 0, [[512, 2], [1, 512]])
            za = sub_ap(zbt[ib], 0, [[512, 2], [1, 512]])
            ma = sub_ap(mub, 0, [[0, 2], [1, 512]])
            nc.vector.tensor_tensor(ya, za, ma, OP.subtract)
            ya2 = sub_ap(y1b[ib], 2 * 512, [[512, 2], [1, 512]])
            za2 = sub_ap(zbt[ib], 2 * 512, [[512, 2], [1, 512]])
            nc.gpsimd.tensor_tensor(ya2, za2, ma, OP.subtract)
            mq, vt, st = musq[ib][:], vart[ib][:], stdt[ib][:]
            nc.scalar.activation(mq, pmu[:], AF.Square)
            nc.vector.scalar_tensor_tensor(vt, psq[:], 1.0, mq,
                                           OP.mult, OP.subtract)
            nc.scalar.activation(st, vt, AF.Sqrt, bias=epsb[:])
            nc.vector.reciprocal(rstdb[ib][:], st)
            for et in range(ET):
                ysl = y1b[ib][:, et * 512:(et + 1) * 512]
                nc.vector.tensor_tensor(ysl, ysl, rstdb[ib][:], OP.mult)
                y8sl = y1f8[ib][:, et * 512:(et + 1) * 512]
                if cast_eng == "act":
                    if et % 2 == 0:
                        nc.scalar.activation(y8sl, ysl, AF.Copy)
                    else:
                        nc.vector.tensor_copy(y8sl, ysl)
                else:
                    nc.gpsimd.tensor_copy(y8sl, ysl)

        def y_ops(ib, zt, yt, yf8=None, dma_out=False, cast_eng="dve",
                  lo=0, hi=512, eng="dve", merged=False):
            if merged:
                # all four feature tiles in two wide strided DVE ops (the
                # [1,w] last dim keeps the 2x bf16 fast path)
                w = hi - lo
                ya = sub_ap(yt[ib], lo, [[512, ET], [1, w]])
                za = sub_ap(zt[ib], lo, [[512, ET], [1, w]])
                ra = sub_ap(rstdb[ib], lo, [[0, ET], [1, w]])
                ca = sub_ap(cmub[ib], lo, [[0, ET], [1, w]])
                nc.vector.tensor_tensor(ya, za, ra, OP.mult)
                nc.vector.tensor_tensor(ya, ya, ca, OP.subtract)
                if yf8 is not None:
                    y8a = sub_ap(yf8[ib], lo, [[512, ET], [1, w]])
                    if cast_eng == "act":
                        nc.scalar.activation(y8a, ya, AF.Copy)
                    elif cast_eng == "pool":
                        nc.gpsimd.tensor_copy(y8a, ya)
                    else:
                        nc.vector.tensor_copy(y8a, ya)
            for et in ([] if merged else range(ET)):
                veng = nc.vector if eng == "dve" else nc.gpsimd
                ysl = yt[ib][:, et * 512 + lo:et * 512 + hi]
                veng.tensor_tensor(
                    ysl, zt[ib][:, et * 512 + lo:et * 512 + hi],
                    rstdb[ib][:, lo:hi], OP.mult)
                veng.tensor_tensor(ysl, ysl, cmub[ib][:, lo:hi],
                                   OP.subtract)
                if yf8 is not None:
                    y8sl = yf8[ib][:, et * 512 + lo:et * 512 + hi]
                    if cast_eng == "act":
                        nc.scalar.activation(y8sl, ysl, AF.Copy)
                    elif cast_eng == "pool":
                        nc.gpsimd.tensor_copy(y8sl, ysl)
                    else:
                        nc.vector.tensor_copy(y8sl, ysl)
            if dma_out:
                # single strided DMA for all four feature blocks
                _r = outT_d.rearrange("(et p) n -> p et n", p=128)
                nc.sync.dma_start(
                    out=bass.AP(tensor=_r.tensor,
                                offset=_r.offset + ib * 512 + lo,
                                ap=[_r.ap[0], _r.ap[1], [1, hi - lo]]),
                    in_=sub_ap(yt[ib], lo, [[512, ET], [1, hi - lo]]))

        # out-projections through the 2-deep sblk ring. The pool-close is a
        # hard sync, so LN1 runs after psD opens (its vector chains then
        # overlap FFN1 PE work instead of gating it).
        # out-proj(0) slots into the PE drain of the attention tail (it
        # only needs attnT[0]); the final flush feeding attnT[1] follows
        tbl_ps = psB.tile([128, 1536], f32, tag="sblk", bufs=2,
                          name="tbl_ps")
        nc.scalar.activation(tbl_ps[:, :1], epsb[:], AF.Sqrt)
        outproj_evict(0, "act", "pool")
        flush_transpose(NQT - 1, atti_q.pop(NQT - 1))
        outproj_evict(1, "dve", "pool")
        if DBG:
            nc.sync.dma_start(out=dbg_at[:], in_=attnT[0][:])
            nc.sync.dma_start(out=dbg_zb[:], in_=zbt[0][:])
            nc.sync.dma_start(out=dbg_q[:], in_=qs[0][:])
            nc.sync.dma_start(out=dbg_k[:], in_=ks[0][:])
        psB_cm.__exit__(None, None, None)
        asm_cm.__exit__(None, None, None)
        probs_cm.__exit__(None, None, None)
        p1_cm.__exit__(None, None, None)

        psD_cm = tc.tile_pool(name="psD", bufs=2, space=PSUM)
        psD = psD_cm.__enter__()

        stats_mm("ln1", zbt, zsqt, 0, pool=psD)
        ln1_chain(0, cast_eng="act")
        stats_mm("ln1", zbt, zsqt, 1, pool=psD)
        ln1_chain(1, cast_eng="pool")

        # keep PE warm while the LN1 massage/y chain runs (no readers, so
        # the ph ring reuses these slots without stalling)
        pwu2 = psD.tile([128, 512], f32, tag="ph", name="pwu2", bufs=4)
        for _ in range(8):
            nc.tensor.matmul(pwu2[:], onesb[:], xtbs[:, :512],
                             start=True, stop=True)

        # ================= Phase 4: FFN =================
        def ffn1(ib, exclude=()):
            for f0 in range(0, FT, 4):
                phs = []
                for ft in range(f0, f0 + 4):
                    ph = psD.tile([128, 512], f32, tag="ph",
                                  name=f"ph{ib}_{ft}", bufs=4)
                    phs.append(ph)
                    nc.tensor.matmul(
                        ph[:],
                        dr_pair(w1, ft * 128, F, 128),
                        dr_pair(y1f8[ib], 0, 512, 512),
                        start=True, stop=False, perf_mode=DR)
                for i, ft in enumerate(range(f0, f0 + 4)):
                    ph = phs[i]
                    nc.tensor.matmul(
                        ph[:],
                        dr_pair(w1, 2 * F + ft * 128, F, 128),
                        dr_pair(y1f8[ib], 2 * 512, 512, 512),
                        start=False, stop=True, perf_mode=DR)
                    hsl = hs[ib][:, ft * 512:(ft + 1) * 512]
                    cands = [g for g in ("act", "dve")
                             if g not in exclude]
                    e = min(cands,
                            key=lambda g: _eng_load[g] + _evict_cost(g, 512))
                    _eng_load[e] += _evict_cost(e, 512)
                    if e == "act":
                        nc.scalar.activation(hsl, ph[:], AF.Relu,
                                             bias=b1s[:, ft:ft + 1])
                    else:
                        nc.vector.scalar_tensor_tensor(
                            hsl, ph[:], b1s[:, ft:ft + 1],
                            sub_ap(zcol, 0, [[0, 512]]),
                            OP.add, OP.max)

        def ffn2(ib, et2s=None, cols=(0, 512), sq_eng="pool"):
            lo, hi = cols
            w = hi - lo
            for et2 in (range(ET) if et2s is None else et2s):
                pf = psD.tile([128, w], f32, tag="ph",
                              name=f"pf{ib}_{et2}_{lo}", bufs=4)
                for p in range(FT // 2):
                    nc.tensor.matmul(
                        pf[:],
                        dr_pair(w2, (2 * p) * D + et2 * 128, D, 128),
                        dr_pair(hs[ib], (2 * p) * 512 + lo, 512, w),
                        start=(p == 0), stop=False, perf_mode=DR)
                # residual: + y1 (bf16 identity matmul)
                nc.tensor.matmul(
                    pf[:], idb2[:],
                    y1b[ib][:, et2 * 512 + lo:et2 * 512 + hi],
                    start=False, stop=True)
                zsl = zbt[ib][:, et2 * 512 + lo:et2 * 512 + hi]
                qsl = zsqt[ib][:, et2 * 512 + lo:et2 * 512 + hi]
                # z2 = pf/(16*256) + b2 (bias via ACT), square on Pool
                nc.scalar.activation(zsl, pf[:], AF.Identity,
                                     bias=b2s[:, et2:et2 + 1],
                                     scale=1.0 / 4096.0)
                if sq_eng == "last" and et2 == ET - 1:
                    sqe = nc.vector
                else:
                    sqe = nc.vector if sq_eng == "dve" else nc.gpsimd
                sqe.tensor_tensor(qsl, zsl, zsl, OP.mult)

        _eng_load.update(dve=0.0, act=0.0, pool=0.0)
        ffn1(0)
        _eng_load.update(dve=0.0, act=0.0, pool=0.0)
        ffn1(1)
        if DBG:
            nc.sync.dma_start(out=dbg_y1[:], in_=y1b[0][:])
            nc.sync.dma_start(out=dbg_hs[:], in_=hs[0][:])
        ffn2(0)
        ffn2(1, et2s=[0])
        stats_mm("ln2", zbt, zsqt, 0, pool=psD)
        ln2_chain("ln2", 0)
        # ib1 token-half-major: all of half 0's feature tiles finish first,
        # so its LN chain hides under half 1's FFN2 PE work; the tail is
        # then just half 1's LN chain
        ffn2(1, et2s=[1, 2, 3], cols=(0, 256))
        ffn2(1, et2s=[1, 2, 3], cols=(256, 512))
        for half in range(2):
            lo, hi = half * 256, (half + 1) * 256
            stats_mm(f"ln2h{half}", zbt, zsqt, 1, lo, hi, pool=psD)
            ln2_chain(f"ln2h{half}", 1, lo, hi, sc=half)

        psD_cm.__exit__(None, None, None)
        persist_cm.__exit__(None, None, None)

    nc.compile()
    return nc


@functools.lru_cache(maxsize=1)
def _program_cached():
    return _build_program()


def host_inputs(x, in_proj_w, in_proj_b, out_proj_w, out_proj_b,
                w1, b1, w2, b2, ln1_g, ln1_b, ln2_g, ln2_b):
    f32 = np.float32
    x = np.asarray(x, f32)
    in_proj_w = np.asarray(in_proj_w, f32)
    out_proj_w = np.asarray(out_proj_w, f32)
    w1 = np.asarray(w1, f32)
    w2 = np.asarray(w2, f32)
    b1 = np.asarray(b1, f32)
    b2 = np.asarray(b2, f32)

    assert np.all(np.asarray(in_proj_b) == 0), "nonzero in_proj_b unsupported"
    assert np.all(np.asarray(out_proj_b) == 0), "nonzero out_proj_b unsupported"
    assert np.all(np.asarray(ln1_g) == 1) and np.all(np.asarray(ln1_b) == 0)
    assert np.all(np.asarray(ln2_g) == 1) and np.all(np.asarray(ln2_b) == 0)

    wq = in_proj_w[:D] * np.float32(1.0 / np.sqrt(DH))
    wk = in_proj_w[D:2 * D]
    wvp = in_proj_w[2 * D:]
    # fp8 weights are scaled into e4m3's normal range; descales are folded
    # into exp-scale, the v ones-column, identity matmuls and eviction scales
    wqkT = np.ascontiguousarray(
        np.concatenate([wq * 2048.0, wk * 256.0], 0).T.astype(F8))
    wvT = np.ascontiguousarray((wvp * 256.0).T.astype(F8))
    woT = np.ascontiguousarray(
        (out_proj_w * 256.0).T.reshape(ET, 128, D).transpose(1, 0, 2)
        .reshape(128, ET * D).astype(F8))
    w1T = np.ascontiguousarray(
        (w1 * 16.0).T.reshape(ET, 128, F).transpose(1, 0, 2)
        .reshape(128, ET * F).astype(F8))
    w2T = np.ascontiguousarray(
        (w2 * 256.0).T.reshape(FT, 128, D).transpose(1, 0, 2)
        .reshape(128, FT * D).astype(F8))
    b1t = np.ascontiguousarray(b1.reshape(FT, 128).T) * np.float32(16.0)
    b2t = np.ascontiguousarray(b2.reshape(ET, 128).T)
    idbo = np.ascontiguousarray((np.eye(128) * 16384.0).astype(BF16))
    idt = np.ascontiguousarray(np.eye(128).astype(BF16))
    idb2 = np.ascontiguousarray((np.eye(128) * 4096.0).astype(BF16))

    idx = np.arange(128)
    tri = {
        0: (idx[:, None] >= idx[None, :]),
        2: (idx[:, None] <= idx[None, :]),
    }

    def mask_for(half):
        m = np.zeros((128, NQT, 2, 128), f32)
        for qt in range(NQT):
            for ji, jt in enumerate((0, 2)):
                v = tri[jt]
                if half == 0 and qt == 0 and jt == 0:
                    v = np.zeros((128, 128), bool)
                if half == 1 and qt == NQT - 1 and jt == 2:
                    v = np.zeros((128, 128), bool)
                m[:, qt, ji, :] = v
        return np.ascontiguousarray(m.reshape(128, NQT * 256).astype(BF16))

    masks_by_half = [mask_for(0), mask_for(1)]

    in_maps = []
    for c in range(NCORES):
        b_idx, half = c // 2, c % 2
        s0 = half * NQ
        xpad = np.zeros((KV, D), f32)
        lo = s0 - WIN
        src_lo, src_hi = max(0, lo), min(S, lo + KV)
        xpad[src_lo - lo:src_hi - lo] = x[b_idx, src_lo:src_hi]
        xT = np.ascontiguousarray(xpad.T)
        in_maps.append({
            "xf8": np.ascontiguousarray(xT.astype(F8)),
            "xtb": np.ascontiguousarray(xT.astype(BF16)),
            "wqkT": wqkT, "wvT": wvT, "woT": woT,
            "w1T": w1T, "w2T": w2T, "b1t": b1t, "b2t": b2t,
            "masks": masks_by_half[half],
            "idbo": idbo, "idb2": idb2, "idt": idt,
        })
    return in_maps


def assemble_output(results):
    out = np.empty((B, S, D), np.float32)
    for c in range(NCORES):
        b_idx, half = c // 2, c % 2
        s0 = half * NQ
        out[b_idx, s0:s0 + NQ] = results[c]["outT"].astype(np.float32).T
    return out


def kernel(x, in_proj_w, in_proj_b, out_proj_w, out_proj_b,
           w1, b1, w2, b2, ln1_g, ln1_b, ln2_g, ln2_b):
    global _last_results
    from concourse.bass_utils import run_bass_kernel_spmd

    nc = _program_cached()
    in_maps = host_inputs(x, in_proj_w, in_proj_b, out_proj_w, out_proj_b,
                          w1, b1, w2, b2, ln1_g, ln1_b, ln2_g, ln2_b)
    trace = bool(int(os.environ.get("TRN_KERNEL_TRACE", "0")))
    try:
        res = run_bass_kernel_spmd(nc, in_maps, list(range(NCORES)),
                                   trace=trace)
    except ModuleNotFoundError:
        res = run_bass_kernel_spmd(nc, in_maps, list(range(NCORES)),
                                   trace=False)
    _last_results = res
    return assemble_output(res.results)

